# revision 32
# baseline (speedup 1.0000x reference)
"""GCNII layer on 8 TRN2 NeuronCores (Bass/Tile).

Strategy: partition nodes (and their incoming edges, bucketed by dst) across
the 8 cores.  Per core, nodes are greedily packed into chunks of 32 output
slots balancing the per-(chunk, quarter) edge counts so each cell fits ONE
128-edge tile; the host lays out the per-edge source-feature messages
(feats[src], a pure integer permutation of the bf16-cast input, padding
rows -> row 0) in tile order so the device STREAMS them with big contiguous
HWDGE DMAs -- no per-edge descriptor generation on the device.

Aggregation runs TRANSPOSED: the raw streamed rows are the PE stationary
operand (bf16, FWL) and a device-generated bf16 SCALE-VALUED one-hot is the
moving operand -- oh[p, s] = 0.9*rsqrt(deg[src_p]*deg[dst_s]) iff edge p
lands in slot s -- so a single matmul per tile applies the edge weight AND
scatters, accumulating h3^T[d, slots] in PSUM with no separate per-edge
fold pass.  The one-hot is built by two narrow DVE passes per stream group
(is_equal against an iota constant, then an in-place broadcast multiply by
the per-edge weight, itself computed on device from the shipped integer
degree products).  The 0.1*initial_features residual joins each PSUM
accumulation via a 0.1*I matmul; the identity-mapped linear combine is a
single wide (I + W^T) bf16 matmul with fused ReLU(0.5*x), written out as
bf16 [D, slots] blocks and un-permuted on the host.  Host-side work is
integer bucketing/layout/permutation; all float math runs on device.
"""

import sys

if "/opt/trn_rl_repo" not in sys.path:
    sys.path.insert(0, "/opt/trn_rl_repo")

from contextlib import ExitStack

import ml_dtypes
import numpy as np

N, E, D, NC = 100000, 1600000, 128, 8
NPC = N // NC            # nodes per core: 12500
ALPHA, BETA = 0.1, 0.5
NSUB = 2                 # tiles per chunk (2 x 128-edge tiles per chunk)
SLOT = 16                # output slots per chunk (one-hot width)
GC = 32                  # chunks per stream group == psum block (512 slots)
BC = 32                  # chunks per psum block (512 slots)

F32 = np.float32
BF16 = ml_dtypes.bfloat16


def _balance_nodes(deg_tot, chunks, cap_load):
    """Greedy 1-dim balancing: assign nodes to `chunks` bins (<=SLOT nodes
    each) minimizing max per-bin edge load."""
    n = deg_tot.shape[0]
    order = np.argsort(-deg_tot, kind="stable")
    loads = np.zeros(chunks, np.int64)
    counts = np.zeros(chunks, np.int64)
    chunk_of = np.empty(n, np.int64)
    slot_of = np.empty(n, np.int64)
    for i in order:
        score = loads + deg_tot[i]
        score[counts >= SLOT] = 1 << 60
        c = int(np.argmin(score))
        chunk_of[i] = c
        slot_of[i] = counts[c]
        counts[c] += 1
        loads[c] += deg_tot[i]
    return chunk_of, slot_of, loads


def _host_prep(features, initial_features, W, src, dst):
    """Integer bucketing/layout prep -> per-core device arrays."""
    src = np.ascontiguousarray(src).astype(np.int64, copy=False)
    dst = np.ascontiguousarray(dst).astype(np.int64, copy=False)
    deg = np.bincount(dst, minlength=N)
    degc = np.maximum(deg, 1).astype(np.int64)
    core_of = dst // NPC
    feats_bf = features.astype(BF16)

    # smallest chunk count (multiple of GC) where every chunk's edges fit
    # its NSUB 128-edge tiles on every core
    cores_tmp = None
    CHUNKS = 800
    while True:
        tmp = []
        ok = True
        for c in range(NC):
            em = core_of == c
            e_src = src[em]
            e_loc = dst[em] - c * NPC
            deg_tot = np.bincount(e_loc, minlength=NPC)
            chunk_of, slot_of, loads = _balance_nodes(
                deg_tot, CHUNKS, NSUB * 128)
            if int(loads.max()) > NSUB * 128 or (CHUNKS * SLOT < NPC):
                ok = False
                break
            tmp.append((e_src, e_loc, chunk_of, slot_of))
        if ok:
            cores_tmp = tmp
            break
        CHUNKS += GC

    cap = 128
    COLS = CHUNKS * NSUB                 # one 128-edge tile per (chunk, sub)
    NG = CHUNKS // GC
    NCALLS = NG * NSUB
    per_core = []
    for c in range(NC):
        e_src, e_loc, chunk_of, slot_of = cores_tmp[c]
        e_chunk = chunk_of[e_loc]
        e_slot = slot_of[e_loc]
        o = np.lexsort((e_src, e_chunk))
        e_src, e_slot, e_chunk = e_src[o], e_slot[o], e_chunk[o]
        e_ddeg = degc[e_loc[o] + c * NPC]    # deg[dst] per (sorted) edge
        e_sdeg = degc[e_src]
        cnt = np.bincount(e_chunk, minlength=CHUNKS)
        # [CHUNKS, NSUB, cap] padded tiles (chunk edges fill tiles in order)
        idx_arr = np.zeros((CHUNKS, NSUB * cap), np.int64)
        slt_arr = np.full((CHUNKS, NSUB * cap), -1.0, F32)  # dst slot or -1
        dsd_arr = np.ones((CHUNKS, NSUB * cap), F32)   # deg[src]*deg[dst]
        starts = np.zeros(CHUNKS, np.int64)
        starts[1:] = np.cumsum(cnt)[:-1]
        pos = np.arange(len(e_src)) - starts[e_chunk]
        idx_arr[e_chunk, pos] = e_src
        slt_arr[e_chunk, pos] = e_slot
        dsd_arr[e_chunk, pos] = (e_sdeg * e_ddeg).astype(F32)
        idx_arr = idx_arr.reshape(CHUNKS, NSUB, cap)
        slt_arr = slt_arr.reshape(CHUNKS, NSUB, cap)
        dsd_arr = dsd_arr.reshape(CHUNKS, NSUB, cap)
        # stream-call order: (group g, sub r, chunk-local cl, part p)
        idx_g = idx_arr.reshape(NG, GC, NSUB, cap)
        idx_flat = idx_g.transpose(0, 2, 1, 3).reshape(-1)
        # per-edge message rows, laid out [call, p, (cl, d)] so each call is
        # one contiguous [128, GC*D] stream tile
        emsg = feats_bf[idx_flat]                       # [COLS*128, D]
        emsg = emsg.reshape(NCALLS, GC, cap, D).transpose(0, 2, 1, 3)
        emsg = np.ascontiguousarray(emsg.reshape(NCALLS * cap, GC * D))
        dsd_g = dsd_arr.reshape(NG, GC, NSUB, cap).transpose(0, 2, 1, 3)
        dsd_dev = np.ascontiguousarray(dsd_g.reshape(COLS, 128).T)
        slt_g = slt_arr.reshape(NG, GC, NSUB, cap).transpose(0, 2, 1, 3)
        slt_dev = np.ascontiguousarray(slt_g.reshape(COLS, 128).T).astype(BF16)

        # node layout tables
        nodelist = np.full((CHUNKS, SLOT), -1, np.int64)
        nodelist[chunk_of, slot_of] = np.arange(NPC)
        glob = np.where(nodelist >= 0, nodelist + c * NPC, -1)
        # initpT blocked: [NBLK*128, BC*SLOT] bf16; block bi rows = feature d,
        # cols = (chunk-local cl, slot s); value = init[glob[bi*BC+cl, s], d]
        NBLK = CHUNKS // BC
        gv = glob.reshape(NBLK, BC * SLOT)
        initpT = np.zeros((NBLK, D, BC * SLOT), F32)
        m = gv >= 0
        for bi in range(NBLK):
            mb = m[bi]
            initpT[bi][:, mb] = initial_features[gv[bi][mb]].T
        initpT = initpT.reshape(NBLK * D, BC * SLOT).astype(BF16)
        per_core.append(
            dict(
                emsg=emsg,
                edsd=dsd_dev,
                eslot=slt_dev,
                initpT=np.ascontiguousarray(initpT),
                glob=glob,
            )
        )
    return per_core, CHUNKS


_BUILD_CACHE = {}


def _build(chunks, nsub=NSUB):
    key = (chunks, nsub)
    if key in _BUILD_CACHE:
        return _BUILD_CACHE[key]
    import concourse.bacc as bacc
    import concourse.bass as bass  # noqa: F401
    import concourse.mybir as mybir
    import concourse.tile as tile

    f32 = mybir.dt.float32
    bf16 = mybir.dt.bfloat16
    Alu = mybir.AluOpType
    Act = mybir.ActivationFunctionType

    COLS = chunks * nsub             # total edge tiles (one per chunk,sub)
    NG = chunks // GC                # stream groups
    NCALLS = NG * nsub
    BW = BC * SLOT                   # psum block width (512)
    HB = GC // BC                    # psum blocks per stream group
    NBLK = chunks // BC              # psum blocks total

    nc = bacc.Bacc("TRN2", target_bir_lowering=False)
    emsg = nc.dram_tensor("emsg", [NCALLS * 128, GC * D], bf16,
                          kind="ExternalInput")
    wt = nc.dram_tensor("wt", [D, D], f32, kind="ExternalInput")
    ident = nc.dram_tensor("ident", [128, 128], f32, kind="ExternalInput")
    iotar = nc.dram_tensor("iotar", [128, GC * SLOT], bf16,
                           kind="ExternalInput")
    edsd = nc.dram_tensor("edsd", [128, COLS], f32, kind="ExternalInput")
    eslot = nc.dram_tensor("eslot", [128, COLS], bf16, kind="ExternalInput")
    initpT = nc.dram_tensor("initpT", [NBLK * 128, BW], bf16,
                            kind="ExternalInput")
    outB = nc.dram_tensor("outB", [NBLK * 128, BW], bf16,
                          kind="ExternalOutput")

    with tile.TileContext(nc) as tc, ExitStack() as ctx:
        const = ctx.enter_context(tc.tile_pool(name="const", bufs=1))
        gpools = [ctx.enter_context(tc.tile_pool(name=f"g{r}", bufs=6))
                  for r in range(nsub)]
        opools = [ctx.enter_context(tc.tile_pool(name=f"o{r}", bufs=4))
                  for r in range(nsub)]
        ipool = ctx.enter_context(tc.tile_pool(name="init", bufs=4))
        hpool = ctx.enter_context(tc.tile_pool(name="h3", bufs=6))
        obpool = ctx.enter_context(tc.tile_pool(name="ob", bufs=3))
        ps_agg = ctx.enter_context(tc.tile_pool(name="psagg", bufs=4,
                                                space="PSUM"))
        ps_mm = ctx.enter_context(tc.tile_pool(name="psmm", bufs=3,
                                               space="PSUM"))

        # scl inputs first: the sqrt/reciprocal chain gates the first ohgen.
        # scl = 0.9 * rsqrt(deg[src]*deg[dst]) = 1/sqrt(x/0.81); compute the
        # first group's slice separately so ohgen(0) is not serialized
        # behind the full-table reciprocal.
        GW = GC * nsub               # scl cols per group
        dsd_sb = const.tile([128, COLS], f32)
        nc.scalar.dma_start(out=dsd_sb[:], in_=edsd[:])
        scl_sb = const.tile([128, COLS], bf16)
        qscale = float(1.0 / ((1.0 - ALPHA) ** 2))
        for lo, hi in ((0, GW), (GW, 5 * GW), (5 * GW, COLS)):
            nc.scalar.activation(dsd_sb[:, lo:hi], dsd_sb[:, lo:hi],
                                 Act.Sqrt, scale=qscale)
            nc.vector.reciprocal(dsd_sb[:, lo:hi], dsd_sb[:, lo:hi])
            nc.scalar.activation(scl_sb[:, lo:hi], dsd_sb[:, lo:hi],
                                 Act.Copy)

        slot_sb = const.tile([128, COLS], bf16)
        nc.scalar.dma_start(out=slot_sb[:], in_=eslot[:])
        iota_sb = const.tile([128, GC * SLOT], bf16)
        nc.scalar.dma_start(out=iota_sb[:], in_=iotar[:])
        wt_sb = const.tile([128, 128], f32)
        nc.scalar.dma_start(out=wt_sb[:], in_=wt[:])
        id_sb = const.tile([128, 128], f32)
        nc.scalar.dma_start(out=id_sb[:], in_=ident[:])

        # W1 = I + W^T (bf16), id01 = 0.1*I (bf16)
        w1_sb = const.tile([128, 128], bf16)
        nc.vector.tensor_tensor(w1_sb[:], wt_sb[:], id_sb[:], Alu.add)

        def issue_streams(g):
            bufs = []
            for r in range(nsub):
                call = g * nsub + r
                bufr = gpools[r].tile([128, GC * D], bf16, name=f"b{r}")
                nc.sync.dma_start(
                    out=bufr[:],
                    in_=emsg[call * 128:(call + 1) * 128, :])
                bufs.append(bufr)
            return bufs

        def issue_ohgen(g):
            ohs = []
            for r in range(nsub):
                call = g * nsub + r
                # oh[p, cl, s] = scl[p, cl] if slot[p, cl] == s else 0
                ohr = opools[r].tile([128, GC * SLOT], bf16, name=f"oh{r}")
                nc.vector.tensor_tensor(
                    ohr[:].rearrange("p (t s) -> p t s", t=GC),
                    slot_sb[:, call * GC:(call + 1) * GC]
                    .unsqueeze(-1).broadcast_to([128, GC, SLOT]),
                    iota_sb[:].rearrange("p (t s) -> p t s", t=GC),
                    Alu.is_equal)
                nc.vector.tensor_tensor(
                    ohr[:].rearrange("p (t s) -> p t s", t=GC),
                    ohr[:].rearrange("p (t s) -> p t s", t=GC),
                    scl_sb[:, call * GC:(call + 1) * GC]
                    .unsqueeze(-1).broadcast_to([128, GC, SLOT]),
                    Alu.mult)
                ohs.append(ohr)
            return ohs

        bufs = issue_streams(0)
        nbufs = issue_streams(1) if NG > 1 else None
        n2bufs = issue_streams(2) if NG > 2 else None
        for g in range(NG):
            ohs = issue_ohgen(g)
            n3bufs = issue_streams(g + 3) if g + 3 < NG else None
            for hb in range(HB):
                bi = g * HB + hb
                itile = ipool.tile([128, BW], bf16, name="itile")
                nc.scalar.dma_start(out=itile[:],
                                    in_=initpT[bi * 128:(bi + 1) * 128, :])
                # 0.1*init residual, pre-scaled while the matmuls run
                it01 = hpool.tile([128, BW], bf16, tag="it01")
                nc.scalar.activation(it01[:], itile[:], Act.Copy,
                                     scale=ALPHA)
                psw = ps_agg.tile([128, BW], f32, space="PSUM", name="psw")
                for cb in range(BC):
                    cl = hb * BC + cb
                    for r in range(nsub):
                        nc.tensor.matmul(
                            psw[:, cb * SLOT:(cb + 1) * SLOT],
                            lhsT=bufs[r][:, cl * 128:(cl + 1) * 128],
                            rhs=ohs[r][:, cl * SLOT:(cl + 1) * SLOT],
                            start=(r == 0),
                            stop=(r == nsub - 1),
                        )
                # epilogue: out = relu(0.5 * (I + W^T) @ (psw + 0.1*init))
                h3 = hpool.tile([128, BW], bf16, tag="h3")
                nc.vector.tensor_tensor(h3[:], psw[:], it01[:], Alu.add)
                pmm = ps_mm.tile([128, BW], f32, space="PSUM", name="pmm")
                nc.tensor.matmul(pmm[:], lhsT=w1_sb[:], rhs=h3[:],
                                 start=True, stop=True)
                obw = obpool.tile([128, BW], bf16)
                nc.scalar.activation(obw[:], pmm[:], Act.Relu, scale=BETA)
                nc.scalar.dma_start(out=outB[bi * 128:(bi + 1) * 128, :],
                                    in_=obw[:])
            bufs = nbufs
            nbufs = n2bufs
            n2bufs = n3bufs

    nc.compile()
    _BUILD_CACHE[key] = nc
    return nc


def _install_ntff_shim():
    """antenv.axon_hooks is absent in this image; shim it and wire the real
    NTFF profiling hook via ctypes so trace=True works under axon."""
    import contextlib
    import ctypes
    import types

    try:
        from antenv import axon_hooks  # noqa: F401
        return
    except ImportError:
        pass
    import antenv

    mod = types.ModuleType("antenv.axon_hooks")
    _hook = [None]
    mod.set_axon_ntff_profile_hook = lambda h: _hook.__setitem__(0, h)
    mod.get_axon_ntff_profile_hook = lambda: _hook[0]
    sys.modules["antenv.axon_hooks"] = mod
    antenv.axon_hooks = mod
    try:
        lib = ctypes.CDLL("/opt/axon/libaxon_pjrt.so")
    except OSError:
        return
    if not hasattr(lib, "axon_start_nrt_profile"):
        return
    lib.axon_start_nrt_profile.argtypes = [
        ctypes.POINTER(ctypes.c_int64),
        ctypes.c_size_t,
    ]
    lib.axon_start_nrt_profile.restype = ctypes.c_int64
    lib.axon_stop_nrt_profile.argtypes = [ctypes.c_char_p]
    lib.axon_stop_nrt_profile.restype = ctypes.c_int64

    @contextlib.contextmanager
    def _hook_cm(output_dir, device_ids):
        import jax

        jax.devices()
        if device_ids:
            ids = (ctypes.c_int64 * len(device_ids))(*device_ids)
            rc = lib.axon_start_nrt_profile(ids, len(device_ids))
        else:
            rc = lib.axon_start_nrt_profile(None, 0)
        if rc != 0:
            raise RuntimeError(f"axon_start_nrt_profile rc={rc}")
        try:
            yield
        finally:
            rc = lib.axon_stop_nrt_profile(output_dir.encode())
            if rc != 0:
                print(f"WARNING: axon_stop_nrt_profile rc={rc}", flush=True)

    mod.set_axon_ntff_profile_hook(_hook_cm)


def _run(inputs, trace=False, trace_cores=None):
    from concourse import bass_utils

    if trace:
        _install_ntff_shim()
    features = np.ascontiguousarray(np.asarray(inputs["features"], dtype=F32))
    initial_features = np.ascontiguousarray(
        np.asarray(inputs["initial_features"], dtype=F32)
    )
    W = np.asarray(inputs["W"], dtype=F32)
    src = np.asarray(inputs["src"])
    dst = np.asarray(inputs["dst"])
    per_core, CHUNKS = _host_prep(features, initial_features, W, src, dst)
    nc = _build(CHUNKS)
    wt_np = np.ascontiguousarray(W.T)
    ident_np = np.eye(128, dtype=F32)
    iota_np = np.ascontiguousarray(
        np.tile(np.arange(SLOT, dtype=F32)[None, :], (128, GC)).astype(BF16))
    in_maps = []
    for c in range(NC):
        pc = per_core[c]
        in_maps.append(
            dict(
                emsg=pc["emsg"],
                wt=wt_np,
                ident=ident_np,
                iotar=iota_np,
                edsd=pc["edsd"],
                eslot=pc["eslot"],
                initpT=pc["initpT"],
            )
        )
    res = bass_utils.run_bass_kernel_spmd(
        nc,
        in_maps,
        core_ids=list(range(NC)),
        trace=trace,
        trace_cores=trace_cores,
    )
    NBLK = CHUNKS // BC
    result = np.empty((N, D), F32)
    for c in range(NC):
        glob = per_core[c]["glob"].reshape(-1)
        ob = np.asarray(res.results[c]["outB"], dtype=F32)
        # outB [NBLK*128, BC*SLOT]: block bi rows=d, cols=(cl, s)
        oc = ob.reshape(NBLK, D, BC * SLOT).transpose(0, 2, 1).reshape(-1, D)
        m = glob >= 0
        result[glob[m]] = oc[m]
    return result, res


def kernel(**inputs):
    return _run(inputs, trace=False)[0]
